# revision 1
# baseline (speedup 1.0000x reference)
"""Trainium2 Bass kernel for segment_reduce (mode='average').

Problem: out[b, s] = mean(input[b, ii:jj], axis=0) for s < lengths[b], else 0,
with (ii, jj) = span_indexes[b, s]. Shapes: input [8, 4096, 768] f32,
lengths [8] i32, span_indexes [8, 512, 2] i32.

Sharding: pure data parallel — batch b -> NeuronCore b (8 cores), no comms.

Primary path (aligned uniform spans: ii = s*w, jj = ii + w, 128 % w == 0,
shared across batches — true for the graded inputs, w = 8): the segment-mean
is a matmul with a periodic block-diagonal weight. Token chunk k (128 tokens,
partitions) contributes to spans [k*128//w, ...) via one of `w` shifted
[128, 128] constant matrices R_r (r = k mod w), entries 1/w. For each s-tile
of 128 spans we accumulate its `w` token chunks into PSUM on the Tensor
engine, then apply the validity mask (per-partition scalar) while copying
PSUM -> SBUF on the Vector engine. Reads x exactly once -> memory bound;
everything except the input DMA is tiny.

Fallback (arbitrary spans): host builds a scaled mask matrix
MT[t, s] = (ii_s <= t < jj_s) * valid_s / (jj_s - ii_s) per batch and the
device does out = MT.T @ x with PSUM accumulation over all 32 token chunks.
"""

import numpy as np

B, T, S, D = 8, 4096, 512, 768
N_CORES = 8
P = 128
K_TILES = T // P  # 32
NT = 384  # matmul moving free-dim tile (<=512 fp32)
S_TILES = S // P  # 4

_cache = {}


def _new_bass():
    import concourse.bacc as bacc

    return bacc.Bacc("TRN2", target_bir_lowering=False, debug=False,
                     num_devices=N_CORES)


def _build_aligned(w):
    """Spans are s*w:(s+1)*w. Each s-tile of 128 spans covers w token chunks.

    x arrives split-precision: xhl[t] = concat(bf16(x[t]), bf16(x[t] - hi)),
    [T, 2D] bf16. Both halves accumulate into the same fp32 PSUM, recovering
    ~16+ mantissa bits while the PE runs at full bf16 rate. 1/w is a power of
    two, so the R weights are bf16-exact.
    """
    import concourse.tile as tile
    from concourse import mybir

    f32 = mybir.dt.float32
    bf16 = mybir.dt.bfloat16
    mult = mybir.AluOpType.mult

    nc = _new_bass()
    x_d = nc.dram_tensor("xhl", [T, 2 * D], bf16, kind="ExternalInput")
    r_d = nc.dram_tensor("rmat", [P, w * P], bf16, kind="ExternalInput")
    sc_d = nc.dram_tensor("scale", [P, S_TILES], f32, kind="ExternalInput")
    y_d = nc.dram_tensor("y", [S, D], f32, kind="ExternalOutput")
    x_ap = x_d.ap()
    y_ap = y_d.ap()

    # DMA granularity: CPD token chunks per transfer. Small enough that the
    # Tensor engine gets a steady stream of work, big enough to amortize DMA
    # descriptor overhead.
    import os

    CPD = int(os.environ.get("SEGRED_CPD", "2"))
    cpd = CPD if w % CPD == 0 else (2 if w % 2 == 0 else 1)
    cpd = min(cpd, w)

    def x_chunks(k0, nch):
        # [p, c, h, d] view of token chunks [k0, k0+nch)
        return x_ap[k0 * P:(k0 + nch) * P, :].rearrange(
            "(c p) (h d) -> p c h d", p=P, h=2)

    with tile.TileContext(nc) as tc:
        with (
            tc.tile_pool(name="xp",
                         bufs=int(os.environ.get("SEGRED_BUFS", "16"))) as xp,
            tc.tile_pool(name="pp", bufs=3, space="PSUM") as pp,
            tc.tile_pool(name="op", bufs=2) as op,
            tc.tile_pool(name="sg", bufs=1) as sg,
        ):
            rb = sg.tile([P, w * P], bf16)
            nc.scalar.dma_start(out=rb[:], in_=r_d.ap())
            sct = sg.tile([P, S_TILES], f32)
            nc.scalar.dma_start(out=sct[:], in_=sc_d.ap())
            for g in range(S_TILES):
                pst = [pp.tile([P, NT], f32, tag=f"ps{nt}", name=f"ps{nt}")
                       for nt in range(D // NT)]
                # chunk grouping: cpd chunks per DMA, but taper the final
                # group so the last DMA->matmul->scale->store chain is short
                groups = [cpd] * (w // cpd)
                if g == S_TILES - 1 and cpd >= 2:
                    # split the last transfer into halves (8 -> [4,2,1,1])
                    tail, rem = [], cpd
                    while rem > 1:
                        tail.append(rem // 2)
                        rem -= rem // 2
                    tail.append(rem)
                    groups[-1:] = tail
                r = 0
                for nch in groups:
                    xk = xp.tile([P, cpd, 2, D], bf16)
                    nc.sync.dma_start(
                        out=xk[:, 0:nch, :, :],
                        in_=x_chunks(g * w + r, nch))
                    for c in range(nch):
                        for h in range(2):
                            for nt in range(D // NT):
                                nc.tensor.matmul(
                                    pst[nt][:],
                                    rb[:, (r + c) * P:(r + c + 1) * P],
                                    xk[:, c, h, nt * NT:(nt + 1) * NT],
                                    start=(r + c == 0 and h == 0),
                                    stop=(r + c == w - 1 and h == 1))
                    r += nch
                ot = op.tile([P, D], f32)
                for nt in range(D // NT):
                    nc.vector.tensor_scalar(
                        out=ot[:, nt * NT:(nt + 1) * NT],
                        in0=pst[nt][:],
                        scalar1=sct[:, g:g + 1], scalar2=None, op0=mult)
                    nc.scalar.dma_start(
                        out=y_ap[g * P:(g + 1) * P, nt * NT:(nt + 1) * NT],
                        in_=ot[:, nt * NT:(nt + 1) * NT])
    nc.compile()
    return nc


def _build_general():
    import concourse.tile as tile
    from concourse import mybir

    f32 = mybir.dt.float32

    nc = _new_bass()
    x_d = nc.dram_tensor("xg", [T, D], f32, kind="ExternalInput")
    m_d = nc.dram_tensor("mt", [T, S], f32, kind="ExternalInput")
    y_d = nc.dram_tensor("yg", [S, D], f32, kind="ExternalOutput")
    x_ap = x_d.ap()
    m_ap = m_d.ap()
    y_ap = y_d.ap()

    with tile.TileContext(nc) as tc:
        with (
            tc.tile_pool(name="xp", bufs=3) as xp,
            tc.tile_pool(name="mp", bufs=3) as mp,
            tc.tile_pool(name="op", bufs=2) as op,
            tc.tile_pool(name="pp", bufs=1, space="PSUM") as pp,
        ):
            ps = [[pp.tile([P, NT], f32, tag=f"ps_{st}_{nt}",
                            name=f"ps_{st}_{nt}")
                   for nt in range(D // NT)] for st in range(S_TILES)]
            for k in range(K_TILES):
                xk = xp.tile([P, D], f32)
                nc.sync.dma_start(out=xk[:], in_=x_ap[k * P:(k + 1) * P, :])
                mk = mp.tile([P, S], f32)
                nc.sync.dma_start(out=mk[:], in_=m_ap[k * P:(k + 1) * P, :])
                for st in range(S_TILES):
                    for nt in range(D // NT):
                        nc.tensor.matmul(
                            ps[st][nt][:],
                            mk[:, st * P:(st + 1) * P],
                            xk[:, nt * NT:(nt + 1) * NT],
                            start=(k == 0), stop=(k == K_TILES - 1))
            for st in range(S_TILES):
                ot = op.tile([P, D], f32)
                for nt in range(D // NT):
                    nc.vector.tensor_copy(
                        out=ot[:, nt * NT:(nt + 1) * NT], in_=ps[st][nt][:])
                nc.scalar.dma_start(
                    out=y_ap[st * P:(st + 1) * P, :], in_=ot[:])
    nc.compile()
    return nc


def _detect_aligned(ii, jj):
    """Return span width w if spans are s*w:(s+1)*w for all batches, with
    128 % w == 0 and w small enough to stage w token chunks in SBUF."""
    if not (np.all(ii == ii[0]) and np.all(jj == jj[0])):
        return None
    i0, j0 = ii[0], jj[0]
    w = int(j0[0] - i0[0])
    # power-of-two width <= 32: P % w == 0 and 1/w is bf16-exact
    if w < 1 or w > 32 or P % w != 0 or (w & (w - 1)) != 0:
        return None
    if S * w > T:
        return None
    s = np.arange(S, dtype=np.int64)
    if np.any(i0 != s * w) or np.any(j0 != s * w + w):
        return None
    return w


def _rmat(w):
    """[128, w*128] f32: column block r is R_r with R_r[t, s'] = (s' ==
    (128*r + t) // w) / w."""
    rb = np.zeros((P, w * P), dtype=np.float32)
    t = np.arange(P)
    for r in range(w):
        sp = (P * r + t) // w  # in [0, 128)
        rb[t, r * P + sp] = 1.0 / w
    return rb


def _run_spmd(nc, in_maps, **kw):
    from concourse.bass_utils import run_bass_kernel_spmd

    last = None
    for _ in range(3):  # device errors can be transient right after attach
        try:
            return run_bass_kernel_spmd(nc, in_maps, list(range(N_CORES)), **kw)
        except Exception as e:  # noqa: BLE001
            last = e
    raise last


def _prepare(input, lengths, span_indexes):
    x = np.asarray(input, dtype=np.float32)
    lengths = np.asarray(lengths).astype(np.int64)
    si = np.asarray(span_indexes).astype(np.int64)
    assert x.shape == (B, T, D), x.shape
    ii, jj = si[..., 0], si[..., 1]
    valid = (np.arange(S)[None, :] < lengths[:, None])  # [B, S]

    w = _detect_aligned(ii, jj)
    if w is not None:
        import os

        import ml_dtypes

        bf16 = ml_dtypes.bfloat16
        key = ("a", w, os.environ.get("SEGRED_CPD", "2"),
               os.environ.get("SEGRED_BUFS", "16"))
        if key not in _cache:
            _cache[key] = _build_aligned(w)
        rb = _rmat(w).astype(bf16)
        xh = x.astype(bf16)  # [B, T, D]
        xl = (x - xh.astype(np.float32)).astype(bf16)
        in_maps = []
        for b in range(B):
            # scale column layout: scale[p, g] masks span s = g*128 + p
            sc = valid[b].astype(np.float32).reshape(S_TILES, P).T
            in_maps.append({
                "xhl": np.ascontiguousarray(
                    np.concatenate([xh[b], xl[b]], axis=1)),
                "rmat": rb,
                "scale": np.ascontiguousarray(sc),
            })
        return _cache[key], in_maps, "y"

    if "g" not in _cache:
        _cache["g"] = _build_general()
    n = np.maximum(jj - ii, 1).astype(np.float32)  # [B, S]
    wgt = valid.astype(np.float32) / n  # [B, S]
    t = np.arange(T)[:, None]  # [T, 1]
    in_maps = []
    for b in range(B):
        mt = ((t >= ii[b][None, :]) & (t < jj[b][None, :]))
        mt = mt.astype(np.float32) * wgt[b][None, :]
        in_maps.append({
            "xg": np.ascontiguousarray(x[b]),
            "mt": np.ascontiguousarray(mt),
        })
    return _cache["g"], in_maps, "yg"


def _assemble(results, out_name):
    return np.ascontiguousarray(
        np.stack([results[b][out_name] for b in range(B)])).astype(np.float32)


def kernel(input, lengths, span_indexes):
    nc, in_maps, out_name = _prepare(input, lengths, span_indexes)
    res = _run_spmd(nc, in_maps)
    return _assemble(res.results, out_name)


def run_traced(input, lengths, span_indexes, trace_cores=None):
    """Test-only entry: run with NTFF tracing, return (output, BassKernelResults)."""
    _install_profile_hook()
    nc, in_maps, out_name, = _prepare(input, lengths, span_indexes)
    res = _run_spmd(nc, in_maps, trace=True, trace_cores=trace_cores)
    return _assemble(res.results, out_name), res


def _install_profile_hook():
    import contextlib
    import ctypes
    import sys
    import types

    if "antenv.axon_hooks" in sys.modules:
        return
    lib = ctypes.CDLL("/opt/axon/libaxon_pjrt.so")
    if not hasattr(lib, "axon_start_nrt_profile"):
        hook = None
    else:
        lib.axon_start_nrt_profile.argtypes = [
            ctypes.POINTER(ctypes.c_int64), ctypes.c_size_t]
        lib.axon_start_nrt_profile.restype = ctypes.c_int64
        lib.axon_stop_nrt_profile.argtypes = [ctypes.c_char_p]
        lib.axon_stop_nrt_profile.restype = ctypes.c_int64

        @contextlib.contextmanager
        def hook(output_dir, device_ids):
            import jax

            jax.devices()
            if device_ids:
                ids = (ctypes.c_int64 * len(device_ids))(*device_ids)
                rc = lib.axon_start_nrt_profile(ids, len(device_ids))
            else:
                rc = lib.axon_start_nrt_profile(None, 0)
            if rc != 0:
                raise RuntimeError(f"axon_start_nrt_profile rc={rc}")
            try:
                yield
            finally:
                n = lib.axon_stop_nrt_profile(str(output_dir).encode())
                print(f"profile: {n} ntff file(s) in {output_dir}",
                      file=sys.stderr)

    mod = types.ModuleType("antenv.axon_hooks")
    mod.get_axon_ntff_profile_hook = lambda: hook
    mod.set_axon_ntff_profile_hook = lambda h: None
    sys.modules["antenv.axon_hooks"] = mod

    import concourse.bass_utils as bu

    bu.upload_artifacts = lambda tmpdir: f"local://{tmpdir}"



# revision 2
# speedup vs baseline: 1.8894x; 1.8894x over previous
"""Trainium2 Bass kernel for segment_reduce (mode='average').

Problem: out[b, s] = mean(input[b, ii:jj], axis=0) for s < lengths[b], else 0,
with (ii, jj) = span_indexes[b, s]. Shapes: input [8, 4096, 768] f32,
lengths [8] i32, span_indexes [8, 512, 2] i32.

Primary path (aligned uniform spans: ii = s*w, jj = ii + w, 128 % w == 0,
shared across batches — true for the graded inputs, w = 8): the segment-mean
is a matmul with a periodic block-diagonal weight. A token chunk of 128
tokens covers q = 128/w whole spans, so the only chunks that matter are the
ones whose spans are valid (s < lengths[b]) — roughly half of them for the
graded lengths. The host packs exactly those chunks (from any batch),
load-balanced across the 8 cores, into one [n_slots*128, D] bf16 tensor per
core. On device, slot r of each group of w chunks is matmul'd with a shifted
[128, 128] constant R_r (entries 1/w) accumulating into a [128, D] PSUM span
tile; the tile is then copied to SBUF as bf16 and streamed out. The host
scatters valid rows back into the full [B, S, D] f32 output.

Accuracy: bf16 input + bf16 output rounding gives ~4e-3 max rel err vs the
f32 reference (gate is 2e-2). PSUM accumulation is f32; 1/w is bf16-exact.

Fallback (arbitrary spans): host builds a scaled mask matrix
MT[t, s] = (ii_s <= t < jj_s) * valid_s / (jj_s - ii_s) per batch and the
device does out = MT.T @ x with PSUM accumulation over all 32 token chunks.
"""

import numpy as np

B, T, S, D = 8, 4096, 512, 768
N_CORES = 8
P = 128
K_TILES = T // P  # 32
NT = 384  # matmul moving free-dim tile (<=512 fp32 PSUM)
S_TILES = S // P  # 4

_cache = {}


def _new_bass():
    import concourse.bacc as bacc

    return bacc.Bacc("TRN2", target_bir_lowering=False, debug=False,
                     num_devices=N_CORES)


def _build_packed(w, n_slots):
    """n_slots chunk slots of 128 tokens each, grouped into ceil(n_slots/w)
    PSUM span tiles (slot r of a group covers span-rows [r*q, (r+1)*q) of the
    group's 128-row tile, q = 128/w). Input bf16, output bf16."""
    import os

    import concourse.tile as tile
    from concourse import mybir

    bf16 = mybir.dt.bfloat16

    q = P // w
    n_full, rem = divmod(n_slots, w)
    group_sizes = [w] * n_full + ([rem] if rem else [])
    n_rows = n_slots * q  # output rows

    nc = _new_bass()
    x_d = nc.dram_tensor("xp", [n_slots * P, D], bf16, kind="ExternalInput")
    r_d = nc.dram_tensor("rmat", [P, w * P], bf16, kind="ExternalInput")
    y_d = nc.dram_tensor("y", [n_rows, D], bf16, kind="ExternalOutput")
    x_ap = x_d.ap()
    y_ap = y_d.ap()

    CPD = int(os.environ.get("SEGRED_CPD", "2"))

    def x_chunks(j0, nch):
        # [p, c, d] view of chunk slots [j0, j0+nch)
        return x_ap[j0 * P:(j0 + nch) * P, :].rearrange(
            "(c p) d -> p c d", p=P)

    with tile.TileContext(nc) as tc:
        with (
            tc.tile_pool(name="xp", bufs=max(2, n_slots)) as xp,
            tc.tile_pool(name="pp", bufs=3, space="PSUM") as pp,
            tc.tile_pool(name="op", bufs=3) as op,
            tc.tile_pool(name="sg", bufs=1) as sg,
        ):
            rb = sg.tile([P, w * P], bf16)
            nc.scalar.dma_start(out=rb[:], in_=r_d.ap())
            base = 0  # slot index of current group start
            ybase = 0  # output row base of current group
            for g, kg in enumerate(group_sizes):
                pst = [pp.tile([P, NT], mybir.dt.float32,
                               tag=f"ps{nt}", name=f"ps{nt}_{g}")
                       for nt in range(D // NT)]
                r = 0
                while r < kg:
                    nch = min(CPD, kg - r)
                    xk = xp.tile([P, nch, D], bf16)
                    nc.sync.dma_start(out=xk[:],
                                      in_=x_chunks(base + r, nch))
                    for c in range(nch):
                        for nt in range(D // NT):
                            nc.tensor.matmul(
                                pst[nt][:],
                                rb[:, (r + c) * P:(r + c + 1) * P],
                                xk[:, c, nt * NT:(nt + 1) * NT],
                                start=(r + c == 0),
                                stop=(r + c == kg - 1))
                    r += nch
                rows = kg * q
                ot = op.tile([P, D], bf16)
                for nt in range(D // NT):
                    nc.vector.tensor_copy(
                        out=ot[0:rows, nt * NT:(nt + 1) * NT],
                        in_=pst[nt][0:rows, :])
                    nc.scalar.dma_start(
                        out=y_ap[ybase:ybase + rows,
                                 nt * NT:(nt + 1) * NT],
                        in_=ot[0:rows, nt * NT:(nt + 1) * NT])
                base += kg
                ybase += rows
    nc.compile()
    return nc


def _build_general():
    import concourse.tile as tile
    from concourse import mybir

    f32 = mybir.dt.float32

    nc = _new_bass()
    x_d = nc.dram_tensor("xg", [T, D], f32, kind="ExternalInput")
    m_d = nc.dram_tensor("mt", [T, S], f32, kind="ExternalInput")
    y_d = nc.dram_tensor("yg", [S, D], f32, kind="ExternalOutput")
    x_ap = x_d.ap()
    m_ap = m_d.ap()
    y_ap = y_d.ap()

    with tile.TileContext(nc) as tc:
        with (
            tc.tile_pool(name="xp", bufs=3) as xp,
            tc.tile_pool(name="mp", bufs=3) as mp,
            tc.tile_pool(name="op", bufs=2) as op,
            tc.tile_pool(name="pp", bufs=1, space="PSUM") as pp,
        ):
            ps = [[pp.tile([P, NT], f32, tag=f"ps_{st}_{nt}",
                            name=f"ps_{st}_{nt}")
                   for nt in range(D // NT)] for st in range(S_TILES)]
            for k in range(K_TILES):
                xk = xp.tile([P, D], f32)
                nc.sync.dma_start(out=xk[:], in_=x_ap[k * P:(k + 1) * P, :])
                mk = mp.tile([P, S], f32)
                nc.sync.dma_start(out=mk[:], in_=m_ap[k * P:(k + 1) * P, :])
                for st in range(S_TILES):
                    for nt in range(D // NT):
                        nc.tensor.matmul(
                            ps[st][nt][:],
                            mk[:, st * P:(st + 1) * P],
                            xk[:, nt * NT:(nt + 1) * NT],
                            start=(k == 0), stop=(k == K_TILES - 1))
            for st in range(S_TILES):
                ot = op.tile([P, D], f32)
                for nt in range(D // NT):
                    nc.vector.tensor_copy(
                        out=ot[:, nt * NT:(nt + 1) * NT], in_=ps[st][nt][:])
                nc.scalar.dma_start(
                    out=y_ap[st * P:(st + 1) * P, :], in_=ot[:])
    nc.compile()
    return nc


def _detect_aligned(ii, jj):
    """Return span width w if spans are s*w:(s+1)*w for all batches, with
    128 % w == 0 and w a power of two (1/w bf16-exact)."""
    if not (np.all(ii == ii[0]) and np.all(jj == jj[0])):
        return None
    i0, j0 = ii[0], jj[0]
    w = int(j0[0] - i0[0])
    if w < 1 or w > 32 or P % w != 0 or (w & (w - 1)) != 0:
        return None
    if S * w > T:
        return None
    s = np.arange(S, dtype=np.int64)
    if np.any(i0 != s * w) or np.any(j0 != s * w + w):
        return None
    return w


def _rmat(w):
    """[128, w*128] f32: column block r is R_r with R_r[t, s'] = (s' ==
    (128*r + t) // w) / w."""
    rb = np.zeros((P, w * P), dtype=np.float32)
    t = np.arange(P)
    for r in range(w):
        sp = (P * r + t) // w  # in [0, 128)
        rb[t, r * P + sp] = 1.0 / w
    return rb


def _run_spmd(nc, in_maps, **kw):
    from concourse.bass_utils import run_bass_kernel_spmd

    last = None
    for _ in range(3):  # device errors can be transient right after attach
        try:
            return run_bass_kernel_spmd(nc, in_maps, list(range(N_CORES)), **kw)
        except Exception as e:  # noqa: BLE001
            last = e
    raise last


def _prepare(input, lengths, span_indexes):
    x = np.asarray(input, dtype=np.float32)
    lengths = np.asarray(lengths).astype(np.int64)
    si = np.asarray(span_indexes).astype(np.int64)
    assert x.shape == (B, T, D), x.shape
    ii, jj = si[..., 0], si[..., 1]
    valid = (np.arange(S)[None, :] < lengths[:, None])  # [B, S]

    w = _detect_aligned(ii, jj)
    if w is not None:
        import os

        import ml_dtypes

        bf16 = ml_dtypes.bfloat16
        q = P // w  # whole spans per 128-token chunk
        # global list of needed chunks: chunk (b, c) covers spans
        # [c*q, (c+1)*q) of batch b; needed iff c*q < lengths[b]
        chunks = [(b, c) for b in range(B)
                  for c in range(-(-int(lengths[b]) // q))]
        n_slots = -(-len(chunks) // N_CORES)
        key = ("p", w, n_slots, os.environ.get("SEGRED_CPD", "2"))
        if key not in _cache:
            _cache[key] = _build_packed(w, n_slots)
        rb = _rmat(w).astype(bf16)
        in_maps = []
        per_core = []
        for k in range(N_CORES):
            mine = chunks[k * n_slots:(k + 1) * n_slots]
            per_core.append(mine)
            xpack = np.zeros((n_slots * P, D), dtype=bf16)
            for j, (b, c) in enumerate(mine):
                xpack[j * P:(j + 1) * P] = x[b, c * P:(c + 1) * P]
            in_maps.append({"xp": xpack, "rmat": rb})

        def assemble(results):
            out = np.zeros((B, S, D), dtype=np.float32)
            for k in range(N_CORES):
                y = np.asarray(results[k]["y"], dtype=np.float32)
                for j, (b, c) in enumerate(per_core[k]):
                    nv = min(q, int(lengths[b]) - c * q)
                    out[b, c * q:c * q + nv] = y[j * q:j * q + nv]
            return out

        return _cache[key], in_maps, assemble

    if "g" not in _cache:
        _cache["g"] = _build_general()
    n = np.maximum(jj - ii, 1).astype(np.float32)  # [B, S]
    wgt = valid.astype(np.float32) / n  # [B, S]
    t = np.arange(T)[:, None]  # [T, 1]
    in_maps = []
    for b in range(B):
        mt = ((t >= ii[b][None, :]) & (t < jj[b][None, :]))
        mt = mt.astype(np.float32) * wgt[b][None, :]
        in_maps.append({
            "xg": np.ascontiguousarray(x[b]),
            "mt": np.ascontiguousarray(mt),
        })

    def assemble(results):
        return np.ascontiguousarray(
            np.stack([results[b]["yg"] for b in range(B)])
        ).astype(np.float32)

    return _cache["g"], in_maps, assemble


def kernel(input, lengths, span_indexes):
    nc, in_maps, assemble = _prepare(input, lengths, span_indexes)
    res = _run_spmd(nc, in_maps)
    return assemble(res.results)


def run_traced(input, lengths, span_indexes, trace_cores=None):
    """Test-only entry: run with NTFF tracing, return (output, BassKernelResults)."""
    _install_profile_hook()
    nc, in_maps, assemble = _prepare(input, lengths, span_indexes)
    res = _run_spmd(nc, in_maps, trace=True, trace_cores=trace_cores)
    return assemble(res.results), res


def _install_profile_hook():
    import contextlib
    import ctypes
    import sys
    import types

    if "antenv.axon_hooks" in sys.modules:
        return
    lib = ctypes.CDLL("/opt/axon/libaxon_pjrt.so")
    if not hasattr(lib, "axon_start_nrt_profile"):
        hook = None
    else:
        lib.axon_start_nrt_profile.argtypes = [
            ctypes.POINTER(ctypes.c_int64), ctypes.c_size_t]
        lib.axon_start_nrt_profile.restype = ctypes.c_int64
        lib.axon_stop_nrt_profile.argtypes = [ctypes.c_char_p]
        lib.axon_stop_nrt_profile.restype = ctypes.c_int64

        @contextlib.contextmanager
        def hook(output_dir, device_ids):
            import jax

            jax.devices()
            if device_ids:
                ids = (ctypes.c_int64 * len(device_ids))(*device_ids)
                rc = lib.axon_start_nrt_profile(ids, len(device_ids))
            else:
                rc = lib.axon_start_nrt_profile(None, 0)
            if rc != 0:
                raise RuntimeError(f"axon_start_nrt_profile rc={rc}")
            try:
                yield
            finally:
                n = lib.axon_stop_nrt_profile(str(output_dir).encode())
                print(f"profile: {n} ntff file(s) in {output_dir}",
                      file=sys.stderr)

    mod = types.ModuleType("antenv.axon_hooks")
    mod.get_axon_ntff_profile_hook = lambda: hook
    mod.set_axon_ntff_profile_hook = lambda h: None
    sys.modules["antenv.axon_hooks"] = mod

    import concourse.bass_utils as bu

    bu.upload_artifacts = lambda tmpdir: f"local://{tmpdir}"
